# revision 7
# baseline (speedup 1.0000x reference)
"""MaxPool3d (kernel=3, stride=2, padding=1) on Trainium2, 8 NeuronCores.

Input  x: (2, 32, 128, 128, 128) f32  ->  Output: (2, 32, 64, 64, 64) f32.

Sharding: the 64 (b, c) slices are data-parallel; each of the 8 cores gets 8
slices, processed as 4 slice-pairs (a pair packs 2 slices into the 128 SBUF
partitions).

Per-core algorithm (separable max pooling W -> H -> D):
  - Load each slice-pair's depth rows into two "parity slabs": even-d rows in
    xE (partition 64*s + d/2), odd-d rows in xO. This makes the final D-axis
    pooling a partition-aligned elementwise max between slabs.
  - W pool (free axis): F = max(x[..., 0::2], x[..., 1::2]);
    F[..., 1:] = max(F[..., 1:], x[..., 1:126:2]).
  - H pool (free axis): G = max(F[:, 0::2], F[:, 1::2]);
    G[:, 1:] = max(G[:, 1:], F[:, 1:126:2]).
  - D pool (partition axis): E = max(G_E, G_O); then the 2*od-1 term via a
    partition-shifted SBUF->SBUF DMA copy of G_O and one more max.
"""

import os
import sys

sys.path.insert(0, "/opt/trn_rl_repo")

import numpy as np

# Shapes (hardcoded per problem spec)
B, C, D, H, W = 2, 32, 128, 128, 128
OD, OH, OW = 64, 64, 64
N_CORES = 8
SLICES_PER_CORE = (B * C) // N_CORES  # 8
PAIRS = SLICES_PER_CORE // 2  # 4
HC = 16  # h rows per load chunk
N_CHUNKS = H // HC

_cache = {}


def _build():
    import concourse.mybir as mybir
    from concourse import bacc
    from concourse.tile import TileContext

    f32 = mybir.dt.float32
    nc = bacc.Bacc()
    x_ext = nc.declare_dram_parameter(
        "x_shard", [SLICES_PER_CORE, D, H, W], f32, isOutput=False
    )
    y_ext = nc.declare_dram_parameter(
        "y_shard", [SLICES_PER_CORE, OD, OH, OW], f32, isOutput=True
    )

    with TileContext(nc) as tc:
        with (
            tc.tile_pool(name="xpool", bufs=2) as xpool,
            tc.tile_pool(name="fpool", bufs=1) as fpool,
            tc.tile_pool(name="gpool", bufs=1) as gpool,
            tc.tile_pool(name="opool", bufs=2) as opool,
        ):
            for p in range(PAIRS):
                s0 = 2 * p
                # ---- load + W pool, one chunk of H rows at a time ----
                F = {}
                X = {}
                for par, name in ((0, "E"), (1, "O")):
                    F[par] = fpool.tile(
                        [128, H, OW], f32, name=f"F{name}", tag=f"F{name}"
                    )
                for c in range(N_CHUNKS):
                    h0 = c * HC
                    for par, name in ((0, "E"), (1, "O")):
                        xt = xpool.tile(
                            [128, HC, W], f32, name=f"x{name}", tag=f"x{name}"
                        )
                        for s in range(2):
                            nc.sync.dma_start(
                                out=xt[64 * s : 64 * s + 64, :, :],
                                in_=x_ext[s0 + s, par : D : 2, h0 : h0 + HC, :],
                            )
                        Fv = F[par][:, h0 : h0 + HC, :]
                        nc.vector.tensor_max(
                            out=Fv, in0=xt[:, :, 0:W:2], in1=xt[:, :, 1:W:2]
                        )
                        nc.vector.tensor_max(
                            out=F[par][:, h0 : h0 + HC, 1:OW],
                            in0=F[par][:, h0 : h0 + HC, 1:OW],
                            in1=xt[:, :, 1 : W - 2 : 2],
                        )
                        X[par] = xt

                # ---- H pool per slab ----
                G = {}
                for par, name in ((0, "E"), (1, "O")):
                    Gt = gpool.tile([128, OH, OW], f32, name=f"G{name}", tag=f"G{name}")
                    nc.vector.tensor_max(
                        out=Gt[:, :, :],
                        in0=F[par][:, 0:H:2, :],
                        in1=F[par][:, 1:H:2, :],
                    )
                    nc.vector.tensor_max(
                        out=Gt[:, 1:OH, :],
                        in0=Gt[:, 1:OH, :],
                        in1=F[par][:, 1 : H - 2 : 2, :],
                    )
                    G[par] = Gt

                # ---- D pool ----
                Et = opool.tile([128, OH, OW], f32, name="Et", tag="Et")
                nc.vector.tensor_max(out=Et[:, :, :], in0=G[0][:, :, :], in1=G[1][:, :, :])

                Gs = gpool.tile([128, OH, OW], f32, name="Gs", tag="Gs")
                nc.sync.dma_start(out=Gs[1:64, :, :], in_=G[1][0:63, :, :])
                nc.sync.dma_start(out=Gs[65:128, :, :], in_=G[1][64:127, :, :])
                # rows 0 and 64 (od=0 of each slice): fill with values already
                # folded into Et (idempotent under max), same-engine copies.
                nc.vector.tensor_copy(out=Gs[0:1, :, :], in_=G[1][0:1, :, :])
                nc.vector.tensor_copy(out=Gs[64:65, :, :], in_=G[1][64:65, :, :])
                nc.vector.tensor_max(out=Et[:, :, :], in0=Et[:, :, :], in1=Gs[:, :, :])

                # ---- store ----
                for s in range(2):
                    nc.sync.dma_start(
                        out=y_ext[s0 + s],
                        in_=Et[64 * s : 64 * s + 64, :, :],
                    )
    nc.compile()
    return nc


def _get_nc():
    if "nc" not in _cache:
        _cache["nc"] = _build()
    return _cache["nc"]


def run(x: np.ndarray, **spmd_kwargs):
    """Run the SPMD kernel; returns the BassKernelResults (for tracing)."""
    from concourse.bass_utils import run_bass_kernel_spmd

    nc = _get_nc()
    xs = np.ascontiguousarray(x, dtype=np.float32).reshape(B * C, D, H, W)
    in_maps = [
        {"x_shard": np.ascontiguousarray(xs[SLICES_PER_CORE * i : SLICES_PER_CORE * (i + 1)])}
        for i in range(N_CORES)
    ]
    return run_bass_kernel_spmd(nc, in_maps, list(range(N_CORES)), **spmd_kwargs)


def kernel(x: np.ndarray) -> np.ndarray:
    res = run(x)
    out = np.stack([res.results[i]["y_shard"] for i in range(N_CORES)])
    return out.reshape(B, C, OD, OH, OW)


# revision 9
# speedup vs baseline: 1.2948x; 1.2948x over previous
"""MaxPool3d (kernel=3, stride=2, padding=1) on Trainium2, 8 NeuronCores.

Input  x: (2, 32, 128, 128, 128) f32  ->  Output: (2, 32, 64, 64, 64) f32.

Sharding: the 64 (b, c) slices are data-parallel; each of the 8 cores gets 8
slices, processed as 4 slice-pairs (a pair packs 2 slices into the 128 SBUF
partitions).

Per-core algorithm (separable max pooling W -> H -> D):
  - Load each slice-pair's depth rows into two "parity slabs": even-d rows in
    xE (partition 64*s + d/2), odd-d rows in xO. This makes the final D-axis
    pooling a partition-aligned elementwise max between slabs.
  - W pool (free axis): F = max(x[..., 0::2], x[..., 1::2]);
    F[..., 1:] = max(F[..., 1:], x[..., 1:126:2]).
  - H pool (free axis): G = max(F[:, 0::2], F[:, 1::2]);
    G[:, 1:] = max(G[:, 1:], F[:, 1:126:2]).  (slab E writes straight into
    the output tile Et)
  - D pool (partition axis): Et = max(Et, G_O); the 2*od-1 term comes from a
    partition-shifted SBUF->SBUF DMA copy of G_O plus one more max.

DMA notes: loads alternate between the two HWDGE rings (nc.sync / nc.scalar)
to halve per-ring FIFO serialization; each load moves a full slice-pair
chunk (2 MiB) in one call.
"""

import os
import sys

sys.path.insert(0, "/opt/trn_rl_repo")

import numpy as np

# Shapes (hardcoded per problem spec)
B, C, D, H, W = 2, 32, 128, 128, 128
OD, OH, OW = 64, 64, 64
N_CORES = 8
SLICES_PER_CORE = (B * C) // N_CORES  # 8
PAIRS = SLICES_PER_CORE // 2  # 4
HC = 32  # h rows per load chunk
N_CHUNKS = H // HC

_cache = {}


def _build():
    import concourse.mybir as mybir
    from concourse import bacc
    from concourse.tile import TileContext

    f32 = mybir.dt.float32
    nc = bacc.Bacc()
    x_ext = nc.declare_dram_parameter(
        "x_shard", [SLICES_PER_CORE, D, H, W], f32, isOutput=False
    )
    y_ext = nc.declare_dram_parameter(
        "y_shard", [SLICES_PER_CORE, OD, OH, OW], f32, isOutput=True
    )

    with TileContext(nc) as tc:
        with (
            tc.tile_pool(name="xpool", bufs=2) as xpool,
            tc.tile_pool(name="fpool", bufs=1) as fpool,
            tc.tile_pool(name="gpool", bufs=1) as gpool,
            tc.tile_pool(name="opool", bufs=2) as opool,
        ):
            dma_rr = [0]

            def load_engine():
                # alternate between the two HWDGE rings
                dma_rr[0] ^= 1
                return nc.sync if dma_rr[0] else nc.scalar

            for p in range(PAIRS):
                s0 = 2 * p
                # ---- load + W pool, one chunk of H rows at a time ----
                F = {}
                for par, name in ((0, "E"), (1, "O")):
                    F[par] = fpool.tile(
                        [128, H, OW], f32, name=f"F{name}", tag=f"F{name}"
                    )
                for c in range(N_CHUNKS):
                    h0 = c * HC
                    for par, name in ((0, "E"), (1, "O")):
                        xt = xpool.tile(
                            [128, HC, W], f32, name=f"x{name}", tag=f"x{name}"
                        )
                        load_engine().dma_start(
                            out=xt[:, :, :],
                            in_=x_ext[s0 : s0 + 2, par : D : 2, h0 : h0 + HC, :],
                        )
                        nc.vector.tensor_max(
                            out=F[par][:, h0 : h0 + HC, :],
                            in0=xt[:, :, 0:W:2],
                            in1=xt[:, :, 1:W:2],
                        )
                        nc.vector.tensor_max(
                            out=F[par][:, h0 : h0 + HC, 1:OW],
                            in0=F[par][:, h0 : h0 + HC, 1:OW],
                            in1=xt[:, :, 1 : W - 2 : 2],
                        )

                # ---- H pool: slab E -> Et (output tile), slab O -> Go ----
                Et = opool.tile([128, OH, OW], f32, name="Et", tag="Et")
                Go = gpool.tile([128, OH, OW], f32, name="Go", tag="Go")
                for par, Gt in ((0, Et), (1, Go)):
                    nc.vector.tensor_max(
                        out=Gt[:, :, :],
                        in0=F[par][:, 0:H:2, :],
                        in1=F[par][:, 1:H:2, :],
                    )
                    nc.vector.tensor_max(
                        out=Gt[:, 1:OH, :],
                        in0=Gt[:, 1:OH, :],
                        in1=F[par][:, 1 : H - 2 : 2, :],
                    )

                # ---- D pool ----
                nc.vector.tensor_max(
                    out=Et[:, :, :], in0=Et[:, :, :], in1=Go[:, :, :]
                )

                Gs = gpool.tile([128, OH, OW], f32, name="Gs", tag="Gs")
                nc.scalar.dma_start(out=Gs[1:64, :, :], in_=Go[0:63, :, :])
                nc.scalar.dma_start(out=Gs[65:128, :, :], in_=Go[64:127, :, :])
                # rows 0 and 64 (od=0 of each slice): fill with rows already
                # folded into Et (idempotent under max).
                nc.scalar.dma_start(
                    out=Gs[0:65:64, :, :], in_=Go[0:65:64, :, :]
                )
                nc.vector.tensor_max(
                    out=Et[:, :, :], in0=Et[:, :, :], in1=Gs[:, :, :]
                )

                # ---- store (one 2 MiB call; balancer splits the 4D out) ----
                nc.sync.dma_start(out=y_ext[s0 : s0 + 2], in_=Et[:, :, :])
    nc.compile()
    return nc


def _get_nc():
    if "nc" not in _cache:
        _cache["nc"] = _build()
    return _cache["nc"]


def run(x: np.ndarray, **spmd_kwargs):
    """Run the SPMD kernel; returns the BassKernelResults (for tracing)."""
    from concourse.bass_utils import run_bass_kernel_spmd

    nc = _get_nc()
    xs = np.ascontiguousarray(x, dtype=np.float32).reshape(B * C, D, H, W)
    in_maps = [
        {"x_shard": np.ascontiguousarray(xs[SLICES_PER_CORE * i : SLICES_PER_CORE * (i + 1)])}
        for i in range(N_CORES)
    ]
    return run_bass_kernel_spmd(nc, in_maps, list(range(N_CORES)), **spmd_kwargs)


def kernel(x: np.ndarray) -> np.ndarray:
    res = run(x)
    out = np.stack([res.results[i]["y_shard"] for i in range(N_CORES)])
    return out.reshape(B, C, OD, OH, OW)
